# revision 38
# baseline (speedup 1.0000x reference)
"""Self-contained Trainium2 (Bass/Tile) attention-layer kernel, 8 NeuronCores.

Problem: nn_AttentionLayer — B=2, S=2048, D=1024, 16 heads x head_dim 64,
fused QKV projections + softmax attention + output projection, fp32 I/O.

Sharding (data + head/tensor parallel): core c handles batch c//4 and the
4-head group c%4 (a 256-wide slice of the model dim).  Q/K/V projection
weights are column-sharded per head group, Wo is row-sharded; each core
produces a partial [S, D] fp16 output and the host reduces the 4 partials
per batch in fp32 and adds the output bias.

The kernel is paced by the ScalarE exp stream (the only exp engine:
16.8M exps/core at 1 elem/lane/cycle); everything else is structured to
hide underneath it:
  * Host ships fp16 inputs transposed, and projection weights
    pre-arranged to their exact SBUF tile layout so every weight DMA is a
    single dense full-bandwidth transfer.  Input-chunk DMAs round-robin
    over the sync/scalar/gpsimd queues (descriptor issue is ~0.6us each).
  * K.T/Q.T per head-pair ([128, S]: heads at rows 0-63 / 64-127) and V'
    (65-strided, ones column per head) by dense tiled matmuls.
  * Attention loops (query-chunk 512, head-pair): per key block the TWO
    heads run as CONCURRENT row-tiled score matmuls (K=64 each,
    tile_position (0,0)/(64,0)) into the two banks of one [128,1024] PSUM
    tile, and ONE FD-1024 exp (exp(s/8 + mask_bias), no max-subtraction;
    the key mask is a per-partition bias shared by both heads) covers the
    pair.  Score tiles are double-buffered so the next block's matmuls
    prefetch under the current exp; PV accumulates U'[65,512] per head
    (row 64 = softmax denominator via the V' ones column).
  * PSUM budget (the binding constraint, 8 banks): scores 2x2 + u' 2x2.
  * Normalization: denominator hops partitions 64->0 with a tensor_copy
    (cross-partition-safe), then reciprocal_approx_fast + GpSimd
    partition_broadcast + tensor_mul, all off the exp critical path.
  * Output projection is emitted sc-major after attention; its
    PSUM->SBUF copies alternate DVE/ScalarE and its DMAs alternate
    sync/gpsimd so no single queue chains the tail.

Measured: 251.3 us HW exec (core 0), rel err 6.1e-4 vs the fp32
reference (baseline this session started from: 389.3 us).
"""

import hashlib
import os
import shutil

import numpy as np

import concourse.bacc as bacc
import concourse.mybir as mybir
import concourse.tile as tile

F16 = mybir.dt.float16
F32 = mybir.dt.float32

D = 1024          # model dim
S = 2048          # sequence length
HD = 64           # head dim
H_CORE = 4        # heads per core
DC = H_CORE * HD  # 256
N_DB = D // 128   # 8 contraction blocks for projections
N_KB = S // 128   # 16 key blocks
N_SC = S // 512   # 4 query chunks of 512
N_QH = 2
QH = S // N_QH    # 1024-query halves for the attention loop

_NEFF_CACHE = os.environ.get("BASS_NEFF_CACHE", "/root/neff_cache")


import re as _re

_TB_RE = _re.compile(rb'"ant_traceback":"(?:[^"\\]|\\.)*"')
_FILE_RE = _re.compile(rb'"filename":"[^"]*","lineno":\d+')


def _normalize_bir(b):
    """Strip caller-dependent debug strings so the cache key is stable across
    call sites (test.py vs the grading harness)."""
    b = _TB_RE.sub(b'"ant_traceback":""', b)
    b = _FILE_RE.sub(b'"filename":"","lineno":0', b)
    return b


def _install_neff_cache():
    """walrus compiles take minutes and the BIR bytes are deterministic:
    cache compiled NEFFs by content hash."""
    import concourse.bass_utils as bu
    import concourse.bass2jax as b2j

    if getattr(bu, "_neff_cache_installed", False):
        return
    try:
        os.makedirs(_NEFF_CACHE, exist_ok=True)
    except OSError:
        return
    orig = bu.compile_bir_kernel

    def cached(bir_json, tmpdir, neff_name="file.neff"):
        raw = bir_json if isinstance(bir_json, bytes) else bir_json.encode()
        h = hashlib.sha256(_normalize_bir(raw)).hexdigest()
        cpath = os.path.join(_NEFF_CACHE, f"{h}.neff")
        if os.path.exists(cpath):
            out = os.path.join(tmpdir, neff_name)
            shutil.copyfile(cpath, out)
            return out
        p = orig(bir_json, tmpdir, neff_name)
        try:
            tmp = cpath + ".tmp"
            shutil.copyfile(p, tmp)
            os.replace(tmp, cpath)
        except OSError:
            pass
        return p

    bu.compile_bir_kernel = cached
    b2j.compile_bir_kernel = cached
    bu._neff_cache_installed = True


def build_program(n_extra=0, num_devices=8):
    """Emit the per-core Tile program.  n_extra=1 appends one contraction row
    to the projections (ones row in x, bias row in w) to realize nonzero
    bq/bk/bv exactly; the harness data has zero biases so the default
    program skips it."""
    DX = D + n_extra
    nc = bacc.Bacc(None, target_bir_lowering=False, debug=False,
                   disable_frame_to_traceback=True, num_devices=num_devices)

    # weights are shipped host-pre-arranged to the SBUF layout so their DMAs
    # are dense full-bandwidth transfers: w[p, db*DC + m] = W.T[db*128+p, m]
    xqT = nc.dram_tensor("xqT", [DX, S], F16, kind="ExternalInput")
    xkT = nc.dram_tensor("xkT", [DX, S], F16, kind="ExternalInput")
    xvT = nc.dram_tensor("xvT", [DX, S], F16, kind="ExternalInput")
    wqT = nc.dram_tensor("wqT", [128, N_DB * DC], F16, kind="ExternalInput")
    wkT = nc.dram_tensor("wkT", [128, N_DB * DC], F16, kind="ExternalInput")
    wvT = nc.dram_tensor("wvT", [128, N_DB * DC], F16, kind="ExternalInput")
    woT = nc.dram_tensor("woT", [128, 2 * D], F16, kind="ExternalInput")
    mb = nc.dram_tensor("mb", [128, N_KB], F32, kind="ExternalInput")
    if n_extra:
        wbT = nc.dram_tensor("wbT", [1, 3 * DC], F16, kind="ExternalInput")
    outT = nc.dram_tensor("outT", [D, S], F16, kind="ExternalOutput")

    with tile.TileContext(nc) as tc:
        with (
            tc.tile_pool(name="weights", bufs=1) as wpool,
            tc.tile_pool(name="xin", bufs=3) as xpool,
            tc.tile_pool(name="qkt", bufs=1) as qkpool,
            tc.tile_pool(name="vp", bufs=1) as vppool,
            tc.tile_pool(name="et", bufs=6) as epool,
            tc.tile_pool(name="ao", bufs=1) as aopool,
            tc.tile_pool(name="div", bufs=2) as divpool,
            tc.tile_pool(name="osb", bufs=3) as opool,
        ):
            ps_mm_ctx = tc.tile_pool(name="ps_mm", bufs=2, space="PSUM")
            ps_mm = ps_mm_ctx.__enter__()
            # ---- static weights / bias tiles ----
            wq_sb = wpool.tile([128, N_DB * DC], F16, tag="wq")
            wk_sb = wpool.tile([128, N_DB * DC], F16, tag="wk")
            wv_sb = wpool.tile([128, N_DB * DC], F16, tag="wv")
            wo_sb = wpool.tile([128, 2 * D], F16, tag="wo")
            mb_sb = wpool.tile([128, N_KB], F32, tag="mb")
            nc.sync.dma_start(out=wk_sb[:], in_=wkT[:, :])
            nc.scalar.dma_start(out=wq_sb[:], in_=wqT[:, :])
            nc.gpsimd.dma_start(out=wv_sb[:], in_=wvT[:, :])
            nc.sync.dma_start(out=wo_sb[:], in_=woT[:, :])
            nc.scalar.dma_start(out=mb_sb[:], in_=mb[:, :])
            if n_extra:
                wx_sb = wpool.tile([1, 3 * DC], F16, tag="wx")
                onerow = wpool.tile([1, S], F16, tag="onerow")
                nc.gpsimd.dma_start(out=wx_sb[:], in_=wbT[:, :])
                nc.gpsimd.dma_start(out=onerow[:], in_=xqT[D:DX, :])

            # ---- K.T / Q.T projections (K first: scores need K blocks) ----
            QT = [qkpool.tile([128, S], F16, tag=f"qt{i}", name=f"qt{i}")
                  for i in range(2)]
            KT = [qkpool.tile([128, S], F16, tag=f"kt{i}", name=f"kt{i}")
                  for i in range(2)]
            for j, (tname, xT, w_sb, dst) in enumerate(
                    (("k", xkT, wk_sb, KT), ("q", xqT, wq_sb, QT))):
                for sc in range(N_SC):
                    xt = [xpool.tile([128, 512], F16, tag=f"x{db}",
                                     name=f"x{tname}{db}") for db in range(N_DB)]
                    for db in range(N_DB):
                        eng = (nc.sync, nc.scalar, nc.gpsimd)[db % 3]
                        eng.dma_start(
                            out=xt[db][:],
                            in_=xT[db * 128:(db + 1) * 128, sc * 512:(sc + 1) * 512])
                    for hc in range(2):
                        ps = ps_mm.tile([128, 512], F32, tag="mm", name="psmm")
                        for db in range(N_DB):
                            nc.tensor.matmul(
                                ps[:],
                                w_sb[:, db * DC + hc * 128: db * DC + hc * 128 + 128],
                                xt[db][:],
                                start=(db == 0), stop=(db == N_DB - 1 and not n_extra),
                            )
                        if n_extra:
                            nc.tensor.matmul(
                                ps[:],
                                wx_sb[0:1, j * DC + hc * 128: j * DC + hc * 128 + 128],
                                onerow[0:1, sc * 512:(sc + 1) * 512],
                                start=False, stop=True,
                            )
                        nc.vector.tensor_copy(
                            out=dst[hc][:, sc * 512:(sc + 1) * 512], in_=ps[:])

            # ---- V projection into V' (65-strided, ones col per head) ----
            VP = [vppool.tile([128, H_CORE * (HD + 1)], F16, tag=f"vp{kb}",
                              name=f"vp{kb}") for kb in range(N_KB)]
            for sc in range(N_SC):
                xt = [xpool.tile([128, 512], F16, tag=f"x{db}",
                                 name=f"xv{db}") for db in range(N_DB)]
                for db in range(N_DB):
                    eng = (nc.sync, nc.scalar, nc.gpsimd)[db % 3]
                    eng.dma_start(
                        out=xt[db][:],
                        in_=xvT[db * 128:(db + 1) * 128, sc * 512:(sc + 1) * 512])
                for kbi in range(4):
                    kb = sc * 4 + kbi
                    ps = ps_mm.tile([128, 512], F32, tag="mm", name="psmm")
                    for db in range(N_DB):
                        nc.tensor.matmul(
                            ps[:, 0:DC],
                            xt[db][:, kbi * 128:(kbi + 1) * 128],
                            wv_sb[:, db * DC:(db + 1) * DC],
                            start=(db == 0), stop=(db == N_DB - 1 and not n_extra),
                        )
                    if n_extra:
                        nc.tensor.matmul(
                            ps[:, 0:DC],
                            onerow[0:1, kb * 128:(kb + 1) * 128],
                            wx_sb[0:1, 2 * DC:3 * DC],
                            start=False, stop=True,
                        )
                    vp3 = VP[kb][:].rearrange("p (g x) -> p g x", x=HD + 1)
                    nc.vector.tensor_copy(
                        out=vp3[:, :, 0:HD],
                        in_=ps[:, 0:DC].rearrange("p (g m) -> p g m", m=HD))
                    nc.gpsimd.memset(vp3[:, :, HD:HD + 1], 1.0)

            ps_mm_ctx.__exit__(None, None, None)
            ps_s_ctx = tc.tile_pool(name="ps_s", bufs=2, space="PSUM")
            ps_s = ps_s_ctx.__enter__()
            ps_u_ctx = tc.tile_pool(name="ps_u", bufs=2, space="PSUM")
            ps_u = ps_u_ctx.__enter__()

            # ---- attention: (query-chunk, head-pair) runs.  The two heads
            # of a pair run as CONCURRENT row-tiled score matmuls (K=64 at
            # array rows 0-63 / 64-127) into the two banks of ONE [128,1024]
            # PSUM tile, and a single FD-1024 exp covers both heads — so the
            # PE work per exp stays well under the ScalarE rate and the exp
            # stream paces the kernel.  U' row 64 is the softmax denominator
            # (V' ones col); the cheap partition-0 reciprocal_approx_fast
            # gets it via a cross-partition copy. ----
            AO = [aopool.tile([128, S], F16, tag=f"ao{i}", name=f"ao{i}")
                  for i in range(2)]
            for qc in range(N_SC):
                q0 = qc * 512
                for hp in range(2):
                    u2 = [ps_u.tile([HD + 1, 512], F32, tag=f"u{x}", name=f"u{x}")
                          for x in range(2)]
                    for kb in range(N_KB):
                        s_ps = ps_s.tile([128, 2 * 512], F32, tag="s", name="s")
                        for hx in range(2):
                            hr = hx * 64
                            nc.tensor.matmul(
                                s_ps[:, hx * 512:(hx + 1) * 512],
                                KT[hp][hr:hr + 64, kb * 128:(kb + 1) * 128],
                                QT[hp][hr:hr + 64, q0:q0 + 512],
                                start=True, stop=True,
                                tile_position=(hr, 0),
                            )
                        et = epool.tile([128, 2 * 512], F16, tag="et", name="et")
                        nc.scalar.activation(
                            et[:], s_ps[:],
                            mybir.ActivationFunctionType.Exp,
                            bias=mb_sb[:, kb:kb + 1], scale=1.0 / np.sqrt(HD),
                        )
                        for hx in range(2):
                            h = hp * 2 + hx
                            nc.tensor.matmul(
                                u2[hx][:],
                                VP[kb][:, h * (HD + 1):(h + 1) * (HD + 1)],
                                et[:, hx * 512:(hx + 1) * 512],
                                start=(kb == 0), stop=(kb == N_KB - 1),
                            )
                    for hx in range(2):
                        hr = hx * 64
                        # denominator row lives on partition 64; the fast
                        # reciprocal only works at partition 0, so hop it
                        # over with a (cheap, cross-partition-safe) copy.
                        dn = divpool.tile([1, 512], F32, tag="dn", name="dn")
                        nc.vector.tensor_copy(out=dn[:], in_=u2[hx][HD:HD + 1, :])
                        r = divpool.tile([1, 512], F32, tag="r", name="r")
                        nc.vector.reciprocal_approx_fast(r[:], dn[:])
                        R = divpool.tile([HD, 512], F32, tag="R", name="R")
                        nc.gpsimd.partition_broadcast(R[:], r[:])
                        nc.vector.tensor_mul(
                            out=AO[hp][hr:hr + 64, q0:q0 + 512],
                            in0=u2[hx][0:HD, :], in1=R[:])

            ps_u_ctx.__exit__(None, None, None)
            ps_s_ctx.__exit__(None, None, None)
            ps_o_ctx = tc.tile_pool(name="ps_o", bufs=3, space="PSUM")
            ps_o = ps_o_ctx.__enter__()

            # ---- output projection (partial; host sums over head groups).
            # Wide [128,1024] tiles (4 MMs + ONE copy + ONE 256KB DMA each)
            # halve the number of chained MM->copy->DMA pipelines in the
            # tail; scp-outer so the earliest-ready query half goes first ----
            for scp in range(2):
                for ob in range(D // 128):
                    ps = ps_o.tile([128, 1024], F32, tag="mm", name="pso")
                    for sci in range(2):
                        sc = scp * 2 + sci
                        for cb in range(2):
                            nc.tensor.matmul(
                                ps[:, sci * 512:(sci + 1) * 512],
                                wo_sb[:, cb * D + ob * 128: cb * D + ob * 128 + 128],
                                AO[cb][:, sc * 512:(sc + 1) * 512],
                                start=(cb == 0), stop=(cb == 1),
                            )
                    ot = opool.tile([128, 1024], F16, tag="ot", name="ot")
                    if ob % 2 == 0:
                        nc.vector.tensor_copy(out=ot[:], in_=ps[:])
                    else:
                        nc.scalar.copy(out=ot[:], in_=ps[:])
                    oeng = nc.sync if ob % 2 == 0 else nc.gpsimd
                    oeng.dma_start(
                        out=outT[ob * 128:(ob + 1) * 128,
                                 scp * 1024:(scp + 1) * 1024],
                        in_=ot[:])
            ps_o_ctx.__exit__(None, None, None)

    nc.compile()
    return nc


def make_in_maps(q, k, v, mask, Wq, bq, Wk, bk, Wv, bv, Wo, n_extra):
    """Per-core input dicts. Core c: batch c//4, heads 4*(c%4)..4*(c%4)+4."""
    def prep_x(x):
        xt = np.ascontiguousarray(x.T).astype(np.float16)
        if n_extra:
            xt = np.concatenate([xt, np.ones((1, S), np.float16)], axis=0)
        return xt

    def prep_w(W, sl):
        # pre-arrange to the SBUF tile layout [p, db*DC + m]
        wt = W[sl, :].T.astype(np.float16)  # [D, DC]
        return np.ascontiguousarray(
            wt.reshape(N_DB, 128, DC).transpose(1, 0, 2).reshape(128, N_DB * DC))

    xT = {}
    for b in range(2):
        xT[("q", b)] = prep_x(q[b])
        xT[("k", b)] = prep_x(k[b])
        xT[("v", b)] = prep_x(v[b])
    in_maps = []
    for c in range(8):
        b, hg = c // 4, c % 4
        sl = slice(hg * DC, (hg + 1) * DC)
        mbias = np.where(mask[b, 0, 0, :] != 0, np.float32(-1e30),
                         np.float32(0.0)).astype(np.float32)
        mbias = np.ascontiguousarray(mbias.reshape(N_KB, 128).T)  # [128, N_KB]
        wo = Wo[:, sl].T.astype(np.float16)  # [DC, D]
        im = {
            "xqT": xT[("q", b)],
            "xkT": xT[("k", b)],
            "xvT": xT[("v", b)],
            "wqT": prep_w(Wq, sl),
            "wkT": prep_w(Wk, sl),
            "wvT": prep_w(Wv, sl),
            "woT": np.ascontiguousarray(
                wo.reshape(2, 128, D).transpose(1, 0, 2).reshape(128, 2 * D)),
            "mb": mbias,
        }
        if n_extra:
            im["wbT"] = np.ascontiguousarray(np.concatenate(
                [b_[sl].astype(np.float16) for b_ in (bq, bk, bv)])[None, :])
        in_maps.append(im)
    return in_maps


_PROGRAMS = {}


def _get_program(n_extra):
    if n_extra not in _PROGRAMS:
        _install_neff_cache()
        _PROGRAMS[n_extra] = build_program(n_extra)
    return _PROGRAMS[n_extra]


def run_sharded(inputs, trace=False, trace_cores=None):
    """Build in_maps, run the SPMD kernel on cores 0-7, return (results obj,
    combined full output)."""
    from concourse.bass_utils import run_bass_kernel_spmd

    n_extra = int(any(np.any(inputs[b]) for b in ("bq", "bk", "bv")))
    nc = _get_program(n_extra)
    in_maps = make_in_maps(
        inputs["q"], inputs["k"], inputs["v"], inputs["mask"],
        inputs["Wq"], inputs["bq"], inputs["Wk"], inputs["bk"],
        inputs["Wv"], inputs["bv"], inputs["Wo"], n_extra)
    kwargs = {}
    if trace:
        kwargs["trace"] = True
        if trace_cores is not None:
            kwargs["trace_cores"] = trace_cores
    res = run_bass_kernel_spmd(nc, in_maps, core_ids=list(range(8)), **kwargs)
    out = np.zeros((2, S, D), np.float32)
    for c in range(8):
        out[c // 4] += res.results[c]["outT"].T.astype(np.float32)
    out += inputs["bo"].astype(np.float32)
    return res, out


def kernel(**inputs) -> np.ndarray:
    _, out = run_sharded(inputs)
    return out


# revision 40
# speedup vs baseline: 1.1827x; 1.1827x over previous
"""Self-contained Trainium2 (Bass/Tile) attention-layer kernel, 8 NeuronCores.

Problem: nn_AttentionLayer — B=2, S=2048, D=1024, 16 heads x head_dim 64,
fused QKV projections + softmax attention + output projection, fp32 I/O.

Sharding (data + head/tensor parallel): core c handles batch c//4 and the
4-head group c%4 (a 256-wide slice of the model dim).  Q/K/V projection
weights are column-sharded per head group, Wo is row-sharded; each core
produces a partial [S, D] output (fp16) and the host reduces the 4 partials
per batch in fp32 and adds the output bias.

Per-core dataflow (operands fp16, fp32 PSUM accumulation):
  * Host ships transposed fp16 inputs: xqT/xkT/xvT [D, S], wqT/wkT/wvT
    [D, 256], woT [256, D], and a per-key additive mask bias.
  * K.T / Q.T [128, S] per head-pair and V' (65-strided with a ones column
    per head) by tiled matmuls, emitted densely to keep the PE HAM-warm.
  * scores.T tiles [128 keys, 1024 queries]: the two heads of a pair run
    as CONCURRENT row-tiled matmuls (K=64 each, tile_position (0,0) and
    (64,0)) so the full 128-row array is used.
  * softmax without max-subtraction: exp(s/8 + mask_bias) on ScalarE —
    the kb-loop is ScalarE-bound, so scores/PV matmuls hide underneath.
  * PV: U'[65, 1024] += V'_h.T @ E_h over key blocks; row 64 accumulates
    the softmax denominator.
  * normalization: reciprocal_approx_fast (DVE) + partition_broadcast
    (GpSimd) + tensor_mul, overlapped with the next head-pair's matmuls.
  * out.T [D, S] fp16 partial = woT.T @ attnout.T, with the qh-outer loop
    letting the first half of the output projection overlap the division
    tail of the last attention iteration.
"""

import hashlib
import os
import shutil

import numpy as np

import concourse.bacc as bacc
import concourse.mybir as mybir
import concourse.tile as tile

F16 = mybir.dt.float16
F32 = mybir.dt.float32

D = 1024          # model dim
S = 2048          # sequence length
HD = 64           # head dim
H_CORE = 4        # heads per core
DC = H_CORE * HD  # 256
N_DB = D // 128   # 8 contraction blocks for projections
N_KB = S // 128   # 16 key blocks
N_SC = S // 512   # 4 query chunks of 512
N_QH = 2
QH = S // N_QH    # 1024-query halves for the attention loop

_NEFF_CACHE = os.environ.get("BASS_NEFF_CACHE", "/root/neff_cache")


import re as _re

_TB_RE = _re.compile(rb'"ant_traceback":"(?:[^"\\]|\\.)*"')
_FILE_RE = _re.compile(rb'"filename":"[^"]*","lineno":\d+')


def _normalize_bir(b):
    """Strip caller-dependent debug strings so the cache key is stable across
    call sites (test.py vs the grading harness)."""
    b = _TB_RE.sub(b'"ant_traceback":""', b)
    b = _FILE_RE.sub(b'"filename":"","lineno":0', b)
    return b


def _install_neff_cache():
    """walrus compiles take minutes and the BIR bytes are deterministic:
    cache compiled NEFFs by content hash."""
    import concourse.bass_utils as bu
    import concourse.bass2jax as b2j

    if getattr(bu, "_neff_cache_installed", False):
        return
    try:
        os.makedirs(_NEFF_CACHE, exist_ok=True)
    except OSError:
        return
    orig = bu.compile_bir_kernel

    def cached(bir_json, tmpdir, neff_name="file.neff"):
        raw = bir_json if isinstance(bir_json, bytes) else bir_json.encode()
        h = hashlib.sha256(_normalize_bir(raw)).hexdigest()
        cpath = os.path.join(_NEFF_CACHE, f"{h}.neff")
        if os.path.exists(cpath):
            out = os.path.join(tmpdir, neff_name)
            shutil.copyfile(cpath, out)
            return out
        p = orig(bir_json, tmpdir, neff_name)
        try:
            tmp = cpath + ".tmp"
            shutil.copyfile(p, tmp)
            os.replace(tmp, cpath)
        except OSError:
            pass
        return p

    bu.compile_bir_kernel = cached
    b2j.compile_bir_kernel = cached
    bu._neff_cache_installed = True


def build_program(n_extra=0, num_devices=8):
    """Emit the per-core Tile program.  n_extra=1 appends one contraction row
    to the projections (ones row in x, bias row in w) to realize nonzero
    bq/bk/bv exactly; the harness data has zero biases so the default
    program skips it."""
    DX = D + n_extra
    nc = bacc.Bacc(None, target_bir_lowering=False, debug=False,
                   disable_frame_to_traceback=True, num_devices=num_devices)

    # weights are shipped host-pre-arranged to the SBUF layout so their DMAs
    # are dense full-bandwidth transfers: w[p, db*DC + m] = W.T[db*128+p, m]
    xqT = nc.dram_tensor("xqT", [DX, S], F16, kind="ExternalInput")
    xkT = nc.dram_tensor("xkT", [DX, S], F16, kind="ExternalInput")
    xvT = nc.dram_tensor("xvT", [DX, S], F16, kind="ExternalInput")
    wqT = nc.dram_tensor("wqT", [128, N_DB * DC], F16, kind="ExternalInput")
    wkT = nc.dram_tensor("wkT", [128, N_DB * DC], F16, kind="ExternalInput")
    wvT = nc.dram_tensor("wvT", [128, N_DB * DC], F16, kind="ExternalInput")
    woT = nc.dram_tensor("woT", [128, 2 * D], F16, kind="ExternalInput")
    mb = nc.dram_tensor("mb", [128, N_KB], F32, kind="ExternalInput")
    if n_extra:
        wbT = nc.dram_tensor("wbT", [1, 3 * DC], F16, kind="ExternalInput")
    outT = nc.dram_tensor("outT", [D, S], F16, kind="ExternalOutput")

    with tile.TileContext(nc) as tc:
        with (
            tc.tile_pool(name="weights", bufs=1) as wpool,
            tc.tile_pool(name="xin", bufs=3) as xpool,
            tc.tile_pool(name="qkt", bufs=1) as qkpool,
            tc.tile_pool(name="vp", bufs=1) as vppool,
            tc.tile_pool(name="et", bufs=6) as epool,
            tc.tile_pool(name="ao", bufs=1) as aopool,
            tc.tile_pool(name="div", bufs=2) as divpool,
            tc.tile_pool(name="osb", bufs=6) as opool,
        ):
            ps_mm_ctx = tc.tile_pool(name="ps_mm", bufs=2, space="PSUM")
            ps_mm = ps_mm_ctx.__enter__()
            # ---- static weights / bias tiles ----
            wq_sb = wpool.tile([128, N_DB * DC], F16, tag="wq")
            wk_sb = wpool.tile([128, N_DB * DC], F16, tag="wk")
            wv_sb = wpool.tile([128, N_DB * DC], F16, tag="wv")
            wo_sb = wpool.tile([128, 2 * D], F16, tag="wo")
            mb_sb = wpool.tile([128, N_KB], F32, tag="mb")
            nc.sync.dma_start(out=wk_sb[:], in_=wkT[:, :])
            nc.scalar.dma_start(out=wq_sb[:], in_=wqT[:, :])
            nc.gpsimd.dma_start(out=wv_sb[:], in_=wvT[:, :])
            nc.sync.dma_start(out=wo_sb[:], in_=woT[:, :])
            nc.scalar.dma_start(out=mb_sb[:], in_=mb[:, :])
            if n_extra:
                wx_sb = wpool.tile([1, 3 * DC], F16, tag="wx")
                onerow = wpool.tile([1, S], F16, tag="onerow")
                nc.gpsimd.dma_start(out=wx_sb[:], in_=wbT[:, :])
                nc.gpsimd.dma_start(out=onerow[:], in_=xqT[D:DX, :])

            # ---- K.T / Q.T projections (K first: scores need K blocks) ----
            QT = [qkpool.tile([128, S], F16, tag=f"qt{i}", name=f"qt{i}")
                  for i in range(2)]
            KT = [qkpool.tile([128, S], F16, tag=f"kt{i}", name=f"kt{i}")
                  for i in range(2)]
            for j, (tname, xT, w_sb, dst) in enumerate(
                    (("k", xkT, wk_sb, KT), ("q", xqT, wq_sb, QT))):
                for sc in range(N_SC):
                    xt = [xpool.tile([128, 512], F16, tag=f"x{db}",
                                     name=f"x{tname}{db}") for db in range(N_DB)]
                    for db in range(N_DB):
                        eng = (nc.sync, nc.scalar, nc.gpsimd)[db % 3]
                        eng.dma_start(
                            out=xt[db][:],
                            in_=xT[db * 128:(db + 1) * 128, sc * 512:(sc + 1) * 512])
                    for hc in range(2):
                        ps = ps_mm.tile([128, 512], F32, tag="mm", name="psmm")
                        for db in range(N_DB):
                            nc.tensor.matmul(
                                ps[:],
                                w_sb[:, db * DC + hc * 128: db * DC + hc * 128 + 128],
                                xt[db][:],
                                start=(db == 0), stop=(db == N_DB - 1 and not n_extra),
                            )
                        if n_extra:
                            nc.tensor.matmul(
                                ps[:],
                                wx_sb[0:1, j * DC + hc * 128: j * DC + hc * 128 + 128],
                                onerow[0:1, sc * 512:(sc + 1) * 512],
                                start=False, stop=True,
                            )
                        nc.vector.tensor_copy(
                            out=dst[hc][:, sc * 512:(sc + 1) * 512], in_=ps[:])

            # ---- V projection into V' (65-strided, ones col per head) ----
            VP = [vppool.tile([128, H_CORE * (HD + 1)], F16, tag=f"vp{kb}",
                              name=f"vp{kb}") for kb in range(N_KB)]
            for sc in range(N_SC):
                xt = [xpool.tile([128, 512], F16, tag=f"x{db}",
                                 name=f"xv{db}") for db in range(N_DB)]
                for db in range(N_DB):
                    eng = (nc.sync, nc.scalar, nc.gpsimd)[db % 3]
                    eng.dma_start(
                        out=xt[db][:],
                        in_=xvT[db * 128:(db + 1) * 128, sc * 512:(sc + 1) * 512])
                for kbi in range(4):
                    kb = sc * 4 + kbi
                    ps = ps_mm.tile([128, 512], F32, tag="mm", name="psmm")
                    for db in range(N_DB):
                        nc.tensor.matmul(
                            ps[:, 0:DC],
                            xt[db][:, kbi * 128:(kbi + 1) * 128],
                            wv_sb[:, db * DC:(db + 1) * DC],
                            start=(db == 0), stop=(db == N_DB - 1 and not n_extra),
                        )
                    if n_extra:
                        nc.tensor.matmul(
                            ps[:, 0:DC],
                            onerow[0:1, kb * 128:(kb + 1) * 128],
                            wx_sb[0:1, 2 * DC:3 * DC],
                            start=False, stop=True,
                        )
                    vp3 = VP[kb][:].rearrange("p (g x) -> p g x", x=HD + 1)
                    nc.vector.tensor_copy(
                        out=vp3[:, :, 0:HD],
                        in_=ps[:, 0:DC].rearrange("p (g m) -> p g m", m=HD))
                    nc.gpsimd.memset(vp3[:, :, HD:HD + 1], 1.0)

            ps_mm_ctx.__exit__(None, None, None)
            ps_s_ctx = tc.tile_pool(name="ps_s", bufs=2, space="PSUM")
            ps_s = ps_s_ctx.__enter__()
            ps_u_ctx = tc.tile_pool(name="ps_u", bufs=2, space="PSUM")
            ps_u = ps_u_ctx.__enter__()

            # ---- attention: (query-chunk, head-pair) runs.  The two heads
            # of a pair run as CONCURRENT row-tiled score matmuls (K=64 at
            # array rows 0-63 / 64-127) into the two banks of ONE [128,1024]
            # PSUM tile, and a single FD-1024 exp covers both heads — so the
            # PE work per exp stays well under the ScalarE rate and the exp
            # stream paces the kernel.  U' row 64 is the softmax denominator
            # (V' ones col); the cheap partition-0 reciprocal_approx_fast
            # gets it via a cross-partition copy. ----
            AO = [aopool.tile([128, S], F16, tag=f"ao{i}", name=f"ao{i}")
                  for i in range(2)]
            for qc in range(N_SC):
                q0 = qc * 512
                for hp in range(2):
                    u2 = [ps_u.tile([HD + 1, 512], F32, tag=f"u{x}", name=f"u{x}")
                          for x in range(2)]
                    for kb in range(N_KB):
                        s_ps = ps_s.tile([128, 2 * 512], F32, tag="s", name="s")
                        for hx in range(2):
                            hr = hx * 64
                            nc.tensor.matmul(
                                s_ps[:, hx * 512:(hx + 1) * 512],
                                KT[hp][hr:hr + 64, kb * 128:(kb + 1) * 128],
                                QT[hp][hr:hr + 64, q0:q0 + 512],
                                start=True, stop=True,
                                tile_position=(hr, 0),
                            )
                        et = epool.tile([128, 2 * 512], F16, tag="et", name="et")
                        nc.scalar.activation(
                            et[:], s_ps[:],
                            mybir.ActivationFunctionType.Exp,
                            bias=mb_sb[:, kb:kb + 1], scale=1.0 / np.sqrt(HD),
                        )
                        for hx in range(2):
                            h = hp * 2 + hx
                            nc.tensor.matmul(
                                u2[hx][:],
                                VP[kb][:, h * (HD + 1):(h + 1) * (HD + 1)],
                                et[:, hx * 512:(hx + 1) * 512],
                                start=(kb == 0), stop=(kb == N_KB - 1),
                            )
                    for hx in range(2):
                        hr = hx * 64
                        # denominator row lives on partition 64; the fast
                        # reciprocal only works at partition 0, so hop it
                        # over with a (cheap, cross-partition-safe) copy.
                        dn = divpool.tile([1, 512], F32, tag="dn", name="dn")
                        nc.vector.tensor_copy(out=dn[:], in_=u2[hx][HD:HD + 1, :])
                        r = divpool.tile([1, 512], F32, tag="r", name="r")
                        nc.vector.reciprocal_approx_fast(r[:], dn[:])
                        R = divpool.tile([HD, 512], F32, tag="R", name="R")
                        nc.gpsimd.partition_broadcast(R[:], r[:])
                        nc.vector.tensor_mul(
                            out=AO[hp][hr:hr + 64, q0:q0 + 512],
                            in0=u2[hx][0:HD, :], in1=R[:])

            ps_u_ctx.__exit__(None, None, None)
            ps_s_ctx.__exit__(None, None, None)
            ps_o_ctx = tc.tile_pool(name="ps_o", bufs=6, space="PSUM")
            ps_o = ps_o_ctx.__enter__()

            # ---- output projection (partial; host sums over head groups);
            # sc-major so the earliest-finished query chunks project first ----
            for sc in range(N_SC):
                for ob in range(D // 128):
                    ps = ps_o.tile([128, 512], F32, tag="mm", name="pso")
                    for cb in range(2):
                        nc.tensor.matmul(
                            ps[:],
                            wo_sb[:, cb * D + ob * 128: cb * D + ob * 128 + 128],
                            AO[cb][:, sc * 512:(sc + 1) * 512],
                            start=(cb == 0), stop=(cb == 1),
                        )
                    ot = opool.tile([128, 512], F16, tag="ot", name="ot")
                    if ob % 2 == 0:
                        nc.vector.tensor_copy(out=ot[:], in_=ps[:])
                    else:
                        nc.scalar.copy(out=ot[:], in_=ps[:])
                    oeng = nc.sync if ob % 2 == 0 else nc.gpsimd
                    oeng.dma_start(
                        out=outT[ob * 128:(ob + 1) * 128, sc * 512:(sc + 1) * 512],
                        in_=ot[:])
            ps_o_ctx.__exit__(None, None, None)

    nc.compile()
    return nc


def make_in_maps(q, k, v, mask, Wq, bq, Wk, bk, Wv, bv, Wo, n_extra):
    """Per-core input dicts. Core c: batch c//4, heads 4*(c%4)..4*(c%4)+4."""
    def prep_x(x):
        xt = np.ascontiguousarray(x.T).astype(np.float16)
        if n_extra:
            xt = np.concatenate([xt, np.ones((1, S), np.float16)], axis=0)
        return xt

    def prep_w(W, sl):
        # pre-arrange to the SBUF tile layout [p, db*DC + m]
        wt = W[sl, :].T.astype(np.float16)  # [D, DC]
        return np.ascontiguousarray(
            wt.reshape(N_DB, 128, DC).transpose(1, 0, 2).reshape(128, N_DB * DC))

    xT = {}
    for b in range(2):
        xT[("q", b)] = prep_x(q[b])
        xT[("k", b)] = prep_x(k[b])
        xT[("v", b)] = prep_x(v[b])
    in_maps = []
    for c in range(8):
        b, hg = c // 4, c % 4
        sl = slice(hg * DC, (hg + 1) * DC)
        mbias = np.where(mask[b, 0, 0, :] != 0, np.float32(-1e30),
                         np.float32(0.0)).astype(np.float32)
        mbias = np.ascontiguousarray(mbias.reshape(N_KB, 128).T)  # [128, N_KB]
        wo = Wo[:, sl].T.astype(np.float16)  # [DC, D]
        im = {
            "xqT": xT[("q", b)],
            "xkT": xT[("k", b)],
            "xvT": xT[("v", b)],
            "wqT": prep_w(Wq, sl),
            "wkT": prep_w(Wk, sl),
            "wvT": prep_w(Wv, sl),
            "woT": np.ascontiguousarray(
                wo.reshape(2, 128, D).transpose(1, 0, 2).reshape(128, 2 * D)),
            "mb": mbias,
        }
        if n_extra:
            im["wbT"] = np.ascontiguousarray(np.concatenate(
                [b_[sl].astype(np.float16) for b_ in (bq, bk, bv)])[None, :])
        in_maps.append(im)
    return in_maps


_PROGRAMS = {}


def _get_program(n_extra):
    if n_extra not in _PROGRAMS:
        _install_neff_cache()
        _PROGRAMS[n_extra] = build_program(n_extra)
    return _PROGRAMS[n_extra]


def run_sharded(inputs, trace=False, trace_cores=None):
    """Build in_maps, run the SPMD kernel on cores 0-7, return (results obj,
    combined full output)."""
    from concourse.bass_utils import run_bass_kernel_spmd

    n_extra = int(any(np.any(inputs[b]) for b in ("bq", "bk", "bv")))
    nc = _get_program(n_extra)
    in_maps = make_in_maps(
        inputs["q"], inputs["k"], inputs["v"], inputs["mask"],
        inputs["Wq"], inputs["bq"], inputs["Wk"], inputs["bk"],
        inputs["Wv"], inputs["bv"], inputs["Wo"], n_extra)
    kwargs = {}
    if trace:
        kwargs["trace"] = True
        if trace_cores is not None:
            kwargs["trace_cores"] = trace_cores
    res = run_bass_kernel_spmd(nc, in_maps, core_ids=list(range(8)), **kwargs)
    out = np.zeros((2, S, D), np.float32)
    for c in range(8):
        out[c // 4] += res.results[c]["outT"].T.astype(np.float32)
    out += inputs["bo"].astype(np.float32)
    return res, out


def kernel(**inputs) -> np.ndarray:
    _, out = run_sharded(inputs)
    return out
